# revision 16
# baseline (speedup 1.0000x reference)
"""Multi-head causal attention (B=4, L=2048, D=1024, H=16, dh=64) on 8 TRN2 NeuronCores.

Sharding: core i handles batch b = i//2 and head-group g = i%2 (8 heads each).
No cross-core communication needed: each core computes o[b, :, g*512:(g+1)*512].

Per-core dataflow (all layouts chosen so matmul contraction is on partitions):
  inputs (host-prepared): qT/kT/vT = activations transposed [D, L]; wq/wk/wv [D, 512].
  projections: qwT/kwT [128(2 heads x 64dh), L] bf16 = W.T @ actT (f32r matmuls);
           vw_aug [128(Lk sub), 8*65] bf16 = (actT.T @ Wv | v_mask) per k-subtile,
           with a v_mask column appended per head (gives sum-of-exp for free in PV).
  attention, per (q-tile tau of 512, head-pair hp):
           S^T[k,q] = kwT.T @ qwT per 128-k block (two K=64 heads row-packed in PE),
           P^T = exp(S^T/8) via ScalarE (psum->sbuf, bf16), causal zeroing of diagonal
           blocks via gpsimd affine_select, then oT[65, 512] += vw_aug.T @ P^T
           accumulated over k blocks (row 64 = sum of exp).  oT is transposed back
           via PE transpose; rows are scaled by 1/sumexp on DVE and stored.
  The two phases are interleaved per L-block (k/v/q projections of block tau feed
  attention of q-tile tau) so ScalarE exp work overlaps PE projection work.
v_mask is pre-applied to v on host (and to the ones column via vmask_t on device);
q_mask is applied to the returned output on host.  Masks are {0,1} so this is exact.
"""
import numpy as np
from contextlib import ExitStack

import concourse.bass as bass
import concourse.tile as tile
from concourse import bacc, mybir
from concourse.bass_utils import run_bass_kernel_spmd
from concourse.masks import make_identity

F32 = mybir.dt.float32
F32R = mybir.dt.float32r
BF16 = mybir.dt.bfloat16

L = 2048          # sequence length
D = 1024          # d_model
COLS = 512        # projection columns per core (8 heads x 64)
NKSUB = L // 128  # 16 k-subtiles
NTAU = L // 512   # 4 q-tiles
NHP = 4           # head pairs per core


def _build_kernel(interleave=True, sps_bufs=2, pt_bufs=6):
    nc = bacc.Bacc("TRN2", target_bir_lowering=False, debug=False, num_devices=8)

    qT = nc.dram_tensor("qT", [D, L], F32R, kind="ExternalInput").ap()
    kT = nc.dram_tensor("kT", [D, L], F32R, kind="ExternalInput").ap()
    vT = nc.dram_tensor("vT", [D, L], F32R, kind="ExternalInput").ap()
    wq = nc.dram_tensor("wq", [D, COLS], F32R, kind="ExternalInput").ap()
    wk = nc.dram_tensor("wk", [D, COLS], F32R, kind="ExternalInput").ap()
    wv = nc.dram_tensor("wv", [D, COLS], F32R, kind="ExternalInput").ap()
    vmask_t = nc.dram_tensor("vmask_t", [128, NKSUB], F32, kind="ExternalInput").ap()
    out = nc.dram_tensor("out", [L, COLS], F32, kind="ExternalOutput").ap()

    with tile.TileContext(nc) as tc, ExitStack() as ctx:
        sb = ctx.enter_context(tc.tile_pool(name="sb", bufs=1))
        ps = ctx.enter_context(tc.tile_pool(name="ps", bufs=1, space="PSUM"))

        # ---- persistent SBUF tensors ----
        wq_t = [sb.tile([128, COLS], F32R, tag="w", bufs=24, name=f"wq{d}") for d in range(8)]
        wk_t = [sb.tile([128, COLS], F32R, tag="w", bufs=24, name=f"wk{d}") for d in range(8)]
        wv_t = [sb.tile([128, COLS], F32R, tag="w", bufs=24, name=f"wv{d}") for d in range(8)]
        w_loaded = set()

        def load_weights(tname):
            if tname in w_loaded:
                return
            w_loaded.add(tname)
            wt, src = {"q": (wq_t, wq), "k": (wk_t, wk), "v": (wv_t, wv)}[tname]
            for d in range(8):
                nc.sync.dma_start(wt[d][:], src[d * 128:(d + 1) * 128, :])

        vmask_sb = sb.tile([128, NKSUB], F32, tag="vm")
        nc.sync.dma_start(vmask_sb[:], vmask_t[:])
        ident = sb.tile([128, 128], F32, tag="id")
        make_identity(nc, ident[:])

        # causal masks for the two diagonal k-batches (c = b - 2*tau in {0, 1}):
        # m_c[p, j, q] = 1 if q >= 256c + 128j + p else 0
        mask_c = []
        for c in range(2):
            m = sb.tile([128, 1024], BF16, tag="mask", bufs=2, name=f"mask{c}")
            nc.gpsimd.memset(m[:], 1.0)
            m3 = m[:].rearrange("p (j q) -> p j q", j=2)
            nc.gpsimd.affine_select(
                out=m3, in_=m3, compare_op=mybir.AluOpType.is_ge, fill=0.0,
                base=-256 * c, channel_multiplier=-1,
                pattern=[[-128, 2], [1, 512]])
            mask_c.append(m)

        qwT = [sb.tile([128, L], BF16, tag="qwT", bufs=NHP, name=f"qwT{hp}") for hp in range(NHP)]
        kwT = [sb.tile([128, L], BF16, tag="kwT", bufs=NHP, name=f"kwT{hp}") for hp in range(NHP)]
        vw_aug = [sb.tile([128, 8 * 65], BF16, tag="vwa", bufs=NKSUB, name=f"vwa{u}")
                  for u in range(NKSUB)]

        def proj_block(tname, lb):
            """Project one L-block of 512 for tensor tname in {q, k, v}."""
            load_weights(tname)
            wt, src = {"q": (wq_t, qT), "k": (wk_t, kT), "v": (wv_t, vT)}[tname]
            acts = []
            for d in range(8):
                a = sb.tile([128, 512], F32R, tag="act", bufs=16,
                            name=f"a{tname}{lb}{d}")
                nc.sync.dma_start(a[:], src[d * 128:(d + 1) * 128,
                                            lb * 512:(lb + 1) * 512])
                acts.append(a)
            if tname != "v":
                dst = qwT if tname == "q" else kwT
                for hp in range(NHP):
                    p = ps.tile([128, 512], F32, tag="pj", bufs=2,
                                name=f"pj{tname}{lb}{hp}")
                    for d in range(8):
                        nc.tensor.matmul(p[:],
                                         wt[d][:, hp * 128:(hp + 1) * 128],
                                         acts[d][:],
                                         start=(d == 0), stop=(d == 7),
                                         skip_group_check=True)
                    nc.vector.tensor_copy(dst[hp][:, lb * 512:(lb + 1) * 512], p[:])
            else:
                for ls in range(4):
                    u = lb * 4 + ls
                    p = ps.tile([128, 512], F32, tag="pj", bufs=2, name=f"pjv{u}")
                    for d in range(8):
                        nc.tensor.matmul(p[:],
                                         acts[d][:, ls * 128:(ls + 1) * 128],
                                         wv_t[d][:],
                                         start=(d == 0), stop=(d == 7),
                                         skip_group_check=True)
                    v3d = vw_aug[u][:].rearrange("p (h c) -> p h c", h=8)
                    nc.vector.tensor_copy(v3d[:, :, 0:64],
                                          p[:].rearrange("p (h c) -> p h c", h=8))
                    nc.vector.tensor_copy(
                        v3d[:, :, 64:65].squeeze(2),
                        vmask_sb[:, u:u + 1].broadcast_to([128, 8]))

        def attn_tau(tau):
            """Attention for q-tile tau (512 q positions), all head pairs."""
            oo = [sb.tile([128, COLS], F32, tag="oo", bufs=8, name=f"oo{tau}{qs}")
                  for qs in range(4)]
            for hp in range(NHP):
                otp = [ps.tile([65, 512], F32, tag="ot", bufs=2,
                               name=f"ot{tau}{hp}{x}") for x in range(2)]
                kmax = 4 * tau + 3  # last k-subtile (causal)
                for b in range(2 * (tau + 1)):  # batches of 2 k-subtiles
                    for half in range(2):
                        h = hp * 2 + half
                        s = ps.tile([128, 1024], F32, tag="sps", bufs=sps_bufs,
                                    name=f"ss{tau}{hp}{b}{half}")
                        for j in range(2):
                            u = 2 * b + j
                            nc.tensor.matmul(
                                s[:, j * 512:(j + 1) * 512],
                                kwT[hp][64 * half:64 * half + 64, u * 128:(u + 1) * 128],
                                qwT[hp][64 * half:64 * half + 64,
                                        tau * 512:(tau + 1) * 512],
                                start=True, stop=True, skip_group_check=True,
                                tile_position=(64 * half, 0))
                        pt = sb.tile([128, 1024], BF16, tag="pT", bufs=pt_bufs,
                                     name=f"pt{tau}{hp}{b}{half}")
                        nc.scalar.activation(pt[:], s[:],
                                             mybir.ActivationFunctionType.Exp,
                                             scale=0.125)
                        if b >= 2 * tau:  # diagonal blocks: causal zeroing (DVE)
                            nc.vector.tensor_mul(pt[:], pt[:],
                                                 mask_c[b - 2 * tau][:])
                        for j in range(2):
                            u = 2 * b + j
                            nc.tensor.matmul(
                                otp[half][:],
                                vw_aug[u][:, h * 65:h * 65 + 65],
                                pt[:, j * 512:(j + 1) * 512],
                                start=(u == 0), stop=(u == kmax),
                                skip_group_check=True)
                ot_sb = []
                for half in range(2):
                    o1 = sb.tile([65, 512], F32, tag="otsb", bufs=4,
                                 name=f"osb{tau}{hp}{half}")
                    nc.vector.tensor_copy(o1[:], otp[half][:])
                    ot_sb.append(o1)
                for qs in range(4):
                    otr = ps.tile([128, 130], F32, tag="pj", bufs=2,
                                  name=f"otr{tau}{hp}{qs}")
                    for half in range(2):
                        nc.tensor.transpose(
                            otr[:, 65 * half:65 * half + 65],
                            ot_sb[half][:, qs * 128:(qs + 1) * 128],
                            ident[0:65, 0:65])
                    rc = sb.tile([128, 2], F32, tag="rc", bufs=4,
                                 name=f"rc{tau}{hp}{qs}")
                    nc.vector.reciprocal(rc[:], otr[:, 64:130:65])
                    for half in range(2):
                        h = hp * 2 + half
                        nc.vector.tensor_scalar_mul(
                            oo[qs][:, h * 64:(h + 1) * 64],
                            otr[:, 65 * half:65 * half + 64],
                            rc[:, half:half + 1])
            for qs in range(4):
                row = (tau * 4 + qs) * 128
                nc.sync.dma_start(out[row:row + 128, :], oo[qs][:])

        if interleave:
            for tau in range(NTAU):
                proj_block("k", tau)
                proj_block("v", tau)
                proj_block("q", tau)
                attn_tau(tau)
        else:
            for tname in ("k", "v", "q"):
                for lb in range(4):
                    proj_block(tname, lb)
            for tau in range(NTAU):
                attn_tau(tau)

    nc.compile()
    return nc


_NC_CACHE = None


def _get_nc():
    global _NC_CACHE
    if _NC_CACHE is None:
        _NC_CACHE = _build_kernel()
    return _NC_CACHE


def make_in_maps(q, k, v, v_mask, q_mask, Wq, Wk, Wv):
    q = np.asarray(q, np.float32)
    k = np.asarray(k, np.float32)
    v = np.asarray(v, np.float32)
    v_mask = np.asarray(v_mask, np.float32)
    Wq = np.asarray(Wq, np.float32)
    Wk = np.asarray(Wk, np.float32)
    Wv = np.asarray(Wv, np.float32)
    in_maps = []
    for core in range(8):
        b, g = core // 2, core % 2
        cs = slice(g * COLS, (g + 1) * COLS)
        vp = v[b] * v_mask[b][:, None]
        in_maps.append({
            "qT": np.ascontiguousarray(q[b].T),
            "kT": np.ascontiguousarray(k[b].T),
            "vT": np.ascontiguousarray(vp.T),
            "wq": np.ascontiguousarray(Wq[:, cs]),
            "wk": np.ascontiguousarray(Wk[:, cs]),
            "wv": np.ascontiguousarray(Wv[:, cs]),
            "vmask_t": np.ascontiguousarray(v_mask[b].reshape(NKSUB, 128).T),
        })
    return in_maps


def kernel(q, k, v, v_mask, q_mask, Wq, Wk, Wv):
    nc = _get_nc()
    in_maps = make_in_maps(q, k, v, v_mask, q_mask, Wq, Wk, Wv)
    res = run_bass_kernel_spmd(nc, in_maps, core_ids=list(range(8)))
    q_mask = np.asarray(q_mask, np.float32)
    out = np.empty((4, L, 2 * COLS), np.float32)
    for core in range(8):
        b, g = core // 2, core % 2
        out[b, :, g * COLS:(g + 1) * COLS] = res.results[core]["out"]
    out *= q_mask[:, :, None]
    return out
